# revision 7
# baseline (speedup 1.0000x reference)
"""Trainium2 Bass kernel for 16-head causal MHA (B=2, S=2048, D=2048).

Sharding: batch*heads across 8 cores -> core c handles batch c//4,
heads 4*(c%4) .. 4*(c%4)+3 (head_dim 128, 4 heads = 512 cols of the
projection weights). Each core computes its heads' Q/K/V projections,
causal attention, and the partial out-projection  sum_h ctx_h @ wo_h.
Host sums the 4 partials per batch and adds the bias.

All matmuls run on the PE at 1 cycle/row: bf16 for x/Q/K/P operands,
fp32r (rounded fp32) for weights and the N=512 moving operands.
Softmax denominator comes free from a ones-column appended to V, so no
row-max/reduce is needed (scores are ~N(0,1); exp cannot overflow).
"""
import math
import numpy as np
import ml_dtypes

import concourse.bass as bass
import concourse.mybir as mybir
import concourse.tile as tile
from concourse import bacc, masks
from concourse.bass_utils import run_bass_kernel_spmd
from contextlib import ExitStack

f32 = mybir.dt.float32
fp16 = mybir.dt.float16

B, S, D = 2, 2048, 2048
HD = 128            # head dim
NHC = 4             # heads per core
HG = NHC * HD       # 512 weight cols per core
DT = D // 128       # 16 contraction tiles
SB = S // 128       # 16 seq blocks of 128
SC = S // 512       # 4 seq chunks of 512
SCALE = 1.0 / math.sqrt(HD)

# PT (transposed exp-scores) ragged layout: kblock kb covers q columns
# [qa(kb), S) with qa = (kb//4)*512, stored packed in one wide tile.
QA = [(kb // 4) * 512 for kb in range(SB)]
WID = [S - QA[kb] for kb in range(SB)]
OFF = [0] * SB
for _kb in range(1, SB):
    OFF[_kb] = OFF[_kb - 1] + WID[_kb - 1]
PT_COLS = OFF[-1] + WID[-1]  # 20480

_CACHED = {}


def _build():
    nc = bacc.Bacc(trn_type="TRN2", target_bir_lowering=False, debug=False)
    xt_d = nc.dram_tensor("xt", [D, S], fp16, kind="ExternalInput").ap()
    wq_d = nc.dram_tensor("wq", [D, HG], fp16, kind="ExternalInput").ap()
    wk_d = nc.dram_tensor("wk", [D, HG], fp16, kind="ExternalInput").ap()
    wv_d = nc.dram_tensor("wv", [D, HG], fp16, kind="ExternalInput").ap()
    wo_d = nc.dram_tensor("wo", [HG, D], fp16, kind="ExternalInput").ap()
    mask_d = nc.dram_tensor("mask", [128, 4 * 512], fp16, kind="ExternalInput").ap()
    out_d = nc.dram_tensor("out", [S, D], f32, kind="ExternalOutput").ap()

    with tile.TileContext(nc) as tc, ExitStack() as ctx:
        # ---- long-lived pools
        v_p = ctx.enter_context(tc.tile_pool(name="v_p", bufs=SB))
        qt_p = ctx.enter_context(tc.tile_pool(name="qt_p", bufs=NHC))
        kt_p = ctx.enter_context(tc.tile_pool(name="kt_p", bufs=NHC))
        ctxt_p = ctx.enter_context(tc.tile_pool(name="ctxt_p", bufs=NHC))
        const_p = ctx.enter_context(tc.tile_pool(name="const_p", bufs=1))

        mask_sb = const_p.tile([128, 4 * 512], fp16, tag="mask_sb")
        nc.sync.dma_start(out=mask_sb[:], in_=mask_d[:])
        ident = const_p.tile([128, 128], fp16, tag="ident")
        masks.make_identity(nc, ident[:])

        # xT resident in bf16 (phases 0-2 only): rhs of Q/K projections,
        # lhsT of V projection
        xt_scope = ExitStack()
        xt_p = xt_scope.enter_context(tc.tile_pool(name="xt_p", bufs=DT))
        xt = []
        for i in range(DT):
            t = xt_p.tile([128, S], fp16, tag="xt")
            nc.sync.dma_start(out=t[:], in_=xt_d[i * 128:(i + 1) * 128, :])
            xt.append(t)

        # ---- phase 1: V (natural layout [kpos, hd]) for all 4 heads
        # V tile per kblock: [128, 4*130] bf16, head h at cols h*130..h*130+127,
        # ones column at h*130+128 (for the softmax denominator).
        v_tiles = []
        with tc.tile_pool(name="wv_p", bufs=DT) as wv_p:
            wv = []
            for i in range(DT):
                t = wv_p.tile([128, HG], fp16, tag="wv")
                nc.sync.dma_start(out=t[:], in_=wv_d[i * 128:(i + 1) * 128, :])
                wv.append(t)
            with tc.tile_pool(name="psv_p", bufs=2, space="PSUM") as psv_p:
                for kb in range(SB):
                    psv = psv_p.tile([128, HG], f32, tag="psv")
                    for dt_ in range(DT):
                        nc.tensor.matmul(
                            psv[:], xt[dt_][:, kb * 128:(kb + 1) * 128], wv[dt_][:],
                            start=(dt_ == 0), stop=(dt_ == DT - 1))
                    vt = v_p.tile([128, NHC * 130], fp16, tag="vt")
                    vr = vt[:].rearrange("p (h c) -> p h c", c=130)
                    pr = psv[:].rearrange("p (h c) -> p h c", c=128)
                    nc.vector.tensor_copy(vr[:, :, 0:128], pr[:, :, :])
                    nc.vector.memset(vr[:, :, 128:129], 1.0)
                    v_tiles.append(vt)

        # ---- phase 2: QT/KT [hd, S] per head, bf16
        qt_tiles, kt_tiles = [], []
        with tc.tile_pool(name="wq_p", bufs=20) as wq_p, \
             tc.tile_pool(name="wk_p", bufs=20) as wk_p, \
             tc.tile_pool(name="psqk_p", bufs=3, space="PSUM") as psqk_p:
            for h in range(NHC):
                for (w_d, w_pool, dst_pool, dst_list, wtag) in (
                        (wq_d, wq_p, qt_p, qt_tiles, "wqh"),
                        (wk_d, wk_p, kt_p, kt_tiles, "wkh")):
                    dst = dst_pool.tile([128, S], fp16, tag=f"{wtag}dst")
                    wts = []
                    for dt_ in range(DT):
                        wt = w_pool.tile([128, 128], fp16, tag=wtag)
                        nc.sync.dma_start(
                            out=wt[:],
                            in_=w_d[dt_ * 128:(dt_ + 1) * 128,
                                    h * 128:(h + 1) * 128])
                        wts.append(wt)
                    for sc_ in range(SC):
                        ps = psqk_p.tile([128, 512], f32, tag="psqk")
                        for dt_ in range(DT):
                            nc.tensor.matmul(
                                ps[:], wts[dt_][:],
                                xt[dt_][:, sc_ * 512:(sc_ + 1) * 512],
                                start=(dt_ == 0), stop=(dt_ == DT - 1))
                        nc.vector.tensor_copy(dst[:, sc_ * 512:(sc_ + 1) * 512], ps[:])
                    dst_list.append(dst)

        xt_scope.close()

        # ---- phase 3: attention per head (transposed-score space)
        ctxt_tiles = []
        with tc.tile_pool(name="pss_p", bufs=3, space="PSUM") as pss_p, \
             tc.tile_pool(name="psc_p", bufs=2, space="PSUM") as psc_p, \
             tc.tile_pool(name="pst_p", bufs=2, space="PSUM") as pst_p, \
             tc.tile_pool(name="pt_p", bufs=1) as pt_p, \
             tc.tile_pool(name="small_p", bufs=4) as small_p, \
             tc.tile_pool(name="stage_p", bufs=4) as stage_p:
            for h in range(NHC):
                qt_h, kt_h = qt_tiles[h], kt_tiles[h]
                pt = pt_p.tile([128, PT_COLS], fp16, tag="pt")
                # scores^T + exp (+ causal mask on diagonal 512-chunks)
                for kb in range(SB):
                    qa = QA[kb]
                    for qc in range(qa, S, 512):
                        pss = pss_p.tile([128, 512], f32, tag="pss")
                        nc.tensor.matmul(
                            pss[:], kt_h[:, kb * 128:(kb + 1) * 128],
                            qt_h[:, qc:qc + 512], start=True, stop=True)
                        dst = pt[:, OFF[kb] + (qc - qa):OFF[kb] + (qc - qa) + 512]
                        nc.scalar.activation(
                            dst, pss[:], mybir.ActivationFunctionType.Exp,
                            scale=SCALE)
                        if qc == qa:
                            m = kb % 4
                            nc.vector.tensor_mul(
                                dst, dst, mask_sb[:, m * 512:(m + 1) * 512])
                # PV per q-block, ones-column gives softmax denom in col 128
                ctxt_h = ctxt_p.tile([128, S], fp16, tag="ctxt")
                ctxt_tiles.append(ctxt_h)
                for qb in range(SB):
                    psc = psc_p.tile([128, 130], f32, tag="psc")
                    for kb in range(qb + 1):
                        lhs = pt[:, OFF[kb] + (qb * 128 - QA[kb]):
                                 OFF[kb] + (qb * 128 - QA[kb]) + 128]
                        nc.tensor.matmul(
                            psc[:, 0:129], lhs,
                            v_tiles[kb][:, h * 130:h * 130 + 129],
                            start=(kb == 0), stop=(kb == qb))
                    r = small_p.tile([128, 1], f32, tag="recip")
                    nc.vector.reciprocal(r[:], psc[:, 128:129])
                    cn = stage_p.tile([128, 128], fp16, tag="cn")
                    nc.vector.tensor_scalar_mul(cn[:], psc[:, 0:128], r[:])
                    pst = pst_p.tile([128, 128], fp16, tag="pst")
                    nc.tensor.transpose(pst[:], cn[:], ident[:])
                    nc.vector.tensor_copy(
                        ctxt_h[:, qb * 128:(qb + 1) * 128], pst[:])

        # ---- phase 4: out projection, out[s, :] = sum_h ctxT_h[:, s].T @ wo_h
        with tc.tile_pool(name="wo_p", bufs=NHC) as wo_p, \
             tc.tile_pool(name="pso_p", bufs=3, space="PSUM") as pso_p, \
             tc.tile_pool(name="ost_p", bufs=4) as ost_p:
            wo = []
            for h in range(NHC):
                t = wo_p.tile([128, D], fp16, tag="wo")
                nc.sync.dma_start(out=t[:], in_=wo_d[h * 128:(h + 1) * 128, :])
                wo.append(t)
            for st in range(SB):
                for dc in range(SC):
                    pso = pso_p.tile([128, 512], f32, tag="pso")
                    for h in range(NHC):
                        nc.tensor.matmul(
                            pso[:], ctxt_tiles[h][:, st * 128:(st + 1) * 128],
                            wo[h][:, dc * 512:(dc + 1) * 512],
                            start=(h == 0), stop=(h == NHC - 1))
                    ob = ost_p.tile([128, 512], f32, tag="ob")
                    nc.vector.tensor_copy(ob[:], pso[:])
                    nc.sync.dma_start(
                        out=out_d[st * 128:(st + 1) * 128, dc * 512:(dc + 1) * 512],
                        in_=ob[:])
    nc.compile()
    return nc


def _make_mask():
    m = np.zeros((128, 4 * 512), dtype=np.float32)
    for mi in range(4):
        kl = np.arange(128)[:, None]
        ql = np.arange(512)[None, :]
        m[:, mi * 512:(mi + 1) * 512] = (ql >= mi * 128 + kl).astype(np.float32)
    return m.astype(np.float16)


def kernel(x, wq, wk, wv, wo, bo):
    if "nc" not in _CACHED:
        _CACHED["nc"] = _build()
    nc = _CACHED["nc"]
    mask = _make_mask()
    in_maps = []
    for c in range(8):
        b = c // 4
        hs = (c % 4) * HG
        xt = np.ascontiguousarray(np.asarray(x[b], dtype=np.float32).T).astype(
            np.float16)
        in_maps.append({
            "xt": xt,
            "wq": np.ascontiguousarray(np.asarray(wq[:, hs:hs + HG], np.float32)).astype(np.float16),
            "wk": np.ascontiguousarray(np.asarray(wk[:, hs:hs + HG], np.float32)).astype(np.float16),
            "wv": np.ascontiguousarray(np.asarray(wv[:, hs:hs + HG], np.float32)).astype(np.float16),
            "wo": np.ascontiguousarray(np.asarray(wo[hs:hs + HG, :], np.float32)).astype(np.float16),
            "mask": mask,
        })
    res = run_bass_kernel_spmd(nc, in_maps, core_ids=list(range(8)))
    out = np.zeros((B, S, D), dtype=np.float32)
    for b in range(B):
        acc = np.zeros((S, D), dtype=np.float32)
        for c in range(4 * b, 4 * b + 4):
            acc += res.results[c]["out"]
        out[b] = acc + np.asarray(bo, np.float32)[None, :]
    return out


# revision 9
# speedup vs baseline: 1.3295x; 1.3295x over previous
"""Trainium2 Bass kernel for 16-head causal MHA (B=2, S=2048, D=2048).

Sharding: batch*heads across 8 cores -> core c handles batch c//4,
heads 4*(c%4) .. 4*(c%4)+3 (head_dim 128, 4 heads = 512 cols of the
projection weights). Each core computes its heads' Q/K/V projections,
causal attention, and the partial out-projection  sum_h ctx_h @ wo_h.
Host sums the 4 partials per batch and adds the bias.

All matmuls run on the PE at 1 cycle/row: bf16 for x/Q/K/P operands,
fp32r (rounded fp32) for weights and the N=512 moving operands.
Softmax denominator comes free from a ones-column appended to V, so no
row-max/reduce is needed (scores are ~N(0,1); exp cannot overflow).
"""
import math
import numpy as np
import ml_dtypes

import concourse.bass as bass
import concourse.mybir as mybir
import concourse.tile as tile
from concourse import bacc, masks
from concourse.bass_utils import run_bass_kernel_spmd
from contextlib import ExitStack

f32 = mybir.dt.float32
fp16 = mybir.dt.float16

B, S, D = 2, 2048, 2048
HD = 128            # head dim
NHC = 4             # heads per core
HG = NHC * HD       # 512 weight cols per core
DT = D // 128       # 16 contraction tiles
SB = S // 128       # 16 seq blocks of 128
SC = S // 512       # 4 seq chunks of 512
SCALE = 1.0 / math.sqrt(HD)

# PT (transposed exp-scores) ragged layout: kblock kb covers q columns
# [qa(kb), S) with qa = (kb//4)*512, stored packed in one wide tile.
QA = [kb * 128 for kb in range(SB)]
WID = [S - QA[kb] for kb in range(SB)]
OFF = [0] * SB
for _kb in range(1, SB):
    OFF[_kb] = OFF[_kb - 1] + WID[_kb - 1]
PT_COLS = OFF[-1] + WID[-1]  # 20480

_CACHED = {}


def _build():
    nc = bacc.Bacc(trn_type="TRN2", target_bir_lowering=False, debug=False)
    xt_d = nc.dram_tensor("xt", [D, S], fp16, kind="ExternalInput").ap()
    wq_d = nc.dram_tensor("wq", [D, HG], fp16, kind="ExternalInput").ap()
    wk_d = nc.dram_tensor("wk", [D, HG], fp16, kind="ExternalInput").ap()
    wv_d = nc.dram_tensor("wv", [D, HG], fp16, kind="ExternalInput").ap()
    wo_d = nc.dram_tensor("wo", [HG, D], fp16, kind="ExternalInput").ap()
    mask_d = nc.dram_tensor("mask", [128, 512], fp16, kind="ExternalInput").ap()
    out_d = nc.dram_tensor("out", [S, D], f32, kind="ExternalOutput").ap()

    with tile.TileContext(nc) as tc, ExitStack() as ctx:
        # ---- long-lived pools
        v_p = ctx.enter_context(tc.tile_pool(name="v_p", bufs=SB))
        qt_p = ctx.enter_context(tc.tile_pool(name="qt_p", bufs=NHC))
        kt_p = ctx.enter_context(tc.tile_pool(name="kt_p", bufs=NHC))
        ctxt_p = ctx.enter_context(tc.tile_pool(name="ctxt_p", bufs=NHC))
        const_p = ctx.enter_context(tc.tile_pool(name="const_p", bufs=1))

        mask_sb = const_p.tile([128, 512], fp16, tag="mask_sb")
        nc.sync.dma_start(out=mask_sb[:], in_=mask_d[:])
        ident = const_p.tile([128, 128], fp16, tag="ident")
        masks.make_identity(nc, ident[:])

        # xT resident in bf16 (phases 0-2 only): rhs of Q/K projections,
        # lhsT of V projection
        xt_scope = ExitStack()
        xt_p = xt_scope.enter_context(tc.tile_pool(name="xt_p", bufs=DT))
        xt = [None] * DT

        # ---- phase 1: V (natural layout [kpos, hd]) for all 4 heads
        # V tile per kblock: [128, 4*130] bf16, head h at cols h*130..h*130+127,
        # ones column at h*130+128 (for the softmax denominator).
        v_tiles = []
        with tc.tile_pool(name="wv_p", bufs=DT) as wv_p:
            wv = []
            for i in range(DT):
                t = wv_p.tile([128, HG], fp16, tag="wv")
                nc.sync.dma_start(out=t[:], in_=wv_d[i * 128:(i + 1) * 128, :])
                wv.append(t)
                xti = xt_p.tile([128, S], fp16, tag="xt")
                nc.sync.dma_start(out=xti[:], in_=xt_d[i * 128:(i + 1) * 128, :])
                xt[i] = xti
            with tc.tile_pool(name="psv_p", bufs=2, space="PSUM") as psv_p:
                for kb in range(SB):
                    psv = psv_p.tile([128, HG], f32, tag="psv")
                    for dt_ in range(DT):
                        nc.tensor.matmul(
                            psv[:], xt[dt_][:, kb * 128:(kb + 1) * 128], wv[dt_][:],
                            start=(dt_ == 0), stop=(dt_ == DT - 1))
                    vt = v_p.tile([128, NHC * 130], fp16, tag="vt")
                    vr = vt[:].rearrange("p (h c) -> p h c", c=130)
                    pr = psv[:].rearrange("p (h c) -> p h c", c=128)
                    nc.scalar.copy(vr[:, :, 0:128], pr[:, :, :])
                    nc.vector.memset(vr[:, :, 128:129], 1.0)
                    v_tiles.append(vt)

        # ---- phase 2: QT/KT [hd, S] per head, bf16
        qt_tiles, kt_tiles = [], []
        with tc.tile_pool(name="wq_p", bufs=20) as wq_p, \
             tc.tile_pool(name="wk_p", bufs=20) as wk_p, \
             tc.tile_pool(name="psqk_p", bufs=3, space="PSUM") as psqk_p:
            for h in range(NHC):
                for (w_d, w_pool, dst_pool, dst_list, wtag) in (
                        (wq_d, wq_p, qt_p, qt_tiles, "wqh"),
                        (wk_d, wk_p, kt_p, kt_tiles, "wkh")):
                    dst = dst_pool.tile([128, S], fp16, tag=f"{wtag}dst")
                    wts = []
                    for dt_ in range(DT):
                        wt = w_pool.tile([128, 128], fp16, tag=wtag)
                        nc.sync.dma_start(
                            out=wt[:],
                            in_=w_d[dt_ * 128:(dt_ + 1) * 128,
                                    h * 128:(h + 1) * 128])
                        wts.append(wt)
                    for sc_ in range(SC):
                        ps = psqk_p.tile([128, 512], f32, tag="psqk")
                        for dt_ in range(DT):
                            nc.tensor.matmul(
                                ps[:], wts[dt_][:],
                                xt[dt_][:, sc_ * 512:(sc_ + 1) * 512],
                                start=(dt_ == 0), stop=(dt_ == DT - 1))
                        nc.scalar.copy(dst[:, sc_ * 512:(sc_ + 1) * 512], ps[:])
                    dst_list.append(dst)

        xt_scope.close()

        # ---- phase 3: attention per head (transposed-score space)
        ctxt_tiles = []
        with tc.tile_pool(name="pss_p", bufs=3, space="PSUM") as pss_p, \
             tc.tile_pool(name="psc_p", bufs=3, space="PSUM") as psc_p, \
             tc.tile_pool(name="pst_p", bufs=2, space="PSUM") as pst_p, \
             tc.tile_pool(name="pt_p", bufs=2) as pt_p, \
             tc.tile_pool(name="small_p", bufs=8) as small_p, \
             tc.tile_pool(name="stage_p", bufs=18) as stage_p:
            for h in range(NHC):
                qt_h, kt_h = qt_tiles[h], kt_tiles[h]
                pt = pt_p.tile([128, PT_COLS], fp16, tag="pt")
                # scores^T + exp (+ causal mask on diagonal 512-chunks)
                for kb in range(SB):
                    qa = QA[kb]
                    qc = qa
                    while qc < S:
                        w = min(512 - (qc % 512), S - qc)
                        pss = pss_p.tile([128, 512], f32, tag="pss")
                        nc.tensor.matmul(
                            pss[:, 0:w], kt_h[:, kb * 128:(kb + 1) * 128],
                            qt_h[:, qc:qc + w], start=True, stop=True)
                        dst = pt[:, OFF[kb] + (qc - qa):OFF[kb] + (qc - qa) + w]
                        nc.scalar.activation(
                            dst, pss[:, 0:w], mybir.ActivationFunctionType.Exp,
                            scale=SCALE)
                        if qc == qa:
                            nc.vector.tensor_mul(dst, dst, mask_sb[:, 0:w])
                        qc += w
                # PV per q-block, ones-column gives softmax denom in col 128
                ctxt_h = ctxt_p.tile([128, S], fp16, tag="ctxt")
                ctxt_tiles.append(ctxt_h)
                cns = []
                for qb in range(SB):
                    psc = psc_p.tile([128, 130], f32, tag="psc")
                    for kb in range(qb + 1):
                        lhs = pt[:, OFF[kb] + (qb - kb) * 128:
                                 OFF[kb] + (qb - kb) * 128 + 128]
                        nc.tensor.matmul(
                            psc[:, 0:129], lhs,
                            v_tiles[kb][:, h * 130:h * 130 + 129],
                            start=(kb == 0), stop=(kb == qb))
                    r = small_p.tile([128, 1], f32, tag="recip")
                    nc.vector.reciprocal(r[:], psc[:, 128:129])
                    cn = stage_p.tile([128, 128], fp16, tag="cn")
                    nc.vector.tensor_scalar_mul(cn[:], psc[:, 0:128], r[:])
                    cns.append(cn)
                for qb in range(SB):
                    pst = pst_p.tile([128, 128], fp16, tag="pst")
                    nc.tensor.transpose(pst[:], cns[qb][:], ident[:])
                    nc.vector.tensor_copy(
                        ctxt_h[:, qb * 128:(qb + 1) * 128], pst[:])

        # ---- phase 4: out projection, out[s, :] = sum_h ctxT_h[:, s].T @ wo_h
        with tc.tile_pool(name="wo_p", bufs=NHC) as wo_p, \
             tc.tile_pool(name="pso_p", bufs=3, space="PSUM") as pso_p, \
             tc.tile_pool(name="ost_p", bufs=4) as ost_p:
            wo = []
            for h in range(NHC):
                t = wo_p.tile([128, D], fp16, tag="wo")
                nc.sync.dma_start(out=t[:], in_=wo_d[h * 128:(h + 1) * 128, :])
                wo.append(t)
            for st in range(SB):
                for dc in range(SC):
                    pso = pso_p.tile([128, 512], f32, tag="pso")
                    for h in range(NHC):
                        nc.tensor.matmul(
                            pso[:], ctxt_tiles[h][:, st * 128:(st + 1) * 128],
                            wo[h][:, dc * 512:(dc + 1) * 512],
                            start=(h == 0), stop=(h == NHC - 1))
                    ob = ost_p.tile([128, 512], f32, tag="ob")
                    nc.scalar.copy(ob[:], pso[:])
                    nc.sync.dma_start(
                        out=out_d[st * 128:(st + 1) * 128, dc * 512:(dc + 1) * 512],
                        in_=ob[:])
    nc.compile()
    return nc


def _make_mask():
    kl = np.arange(128)[:, None]
    ql = np.arange(512)[None, :]
    return (ql >= kl).astype(np.float16)


def kernel(x, wq, wk, wv, wo, bo):
    if "nc" not in _CACHED:
        _CACHED["nc"] = _build()
    nc = _CACHED["nc"]
    mask = _make_mask()
    in_maps = []
    for c in range(8):
        b = c // 4
        hs = (c % 4) * HG
        xt = np.ascontiguousarray(np.asarray(x[b], dtype=np.float32).T).astype(
            np.float16)
        in_maps.append({
            "xt": xt,
            "wq": np.ascontiguousarray(np.asarray(wq[:, hs:hs + HG], np.float32)).astype(np.float16),
            "wk": np.ascontiguousarray(np.asarray(wk[:, hs:hs + HG], np.float32)).astype(np.float16),
            "wv": np.ascontiguousarray(np.asarray(wv[:, hs:hs + HG], np.float32)).astype(np.float16),
            "wo": np.ascontiguousarray(np.asarray(wo[hs:hs + HG, :], np.float32)).astype(np.float16),
            "mask": mask,
        })
    res = run_bass_kernel_spmd(nc, in_maps, core_ids=list(range(8)))
    out = np.zeros((B, S, D), dtype=np.float32)
    for b in range(B):
        acc = np.zeros((S, D), dtype=np.float32)
        for c in range(4 * b, 4 * b + 4):
            acc += res.results[c]["out"]
        out[b] = acc + np.asarray(bo, np.float32)[None, :]
    return out


# revision 31
# speedup vs baseline: 22319.1142x; 16787.4297x over previous
"""Trainium2 Bass kernel for 16-head causal MHA (B=2, S=2048, D=2048).

Sharding: batch*heads across 8 cores -> core c handles batch c//4,
heads 4*(c%4) .. 4*(c%4)+3 (head_dim 128, 4 heads = 512 cols of the
projection weights). Each core computes its heads' Q/K/V projections,
causal attention, and the partial out-projection  sum_h ctx_h @ wo_h.
Host sums the 4 partials per batch and adds the bias.

All matmuls run fp16 (1 cycle/row on the PE, f32 PSUM accumulation).
Scores are computed transposed (K @ Q^T) so the softmax denominator
comes free from a ones-column appended to V during the P@V matmul; no
row-max/reduce is needed (scores are ~N(0,1); exp cannot overflow).
ctx blocks are transposed for the out-projection via DMA-XBAR.
"""
import math
import numpy as np
import ml_dtypes

import concourse.bass as bass
import concourse.mybir as mybir
import concourse.tile as tile
from concourse import bacc, masks
from concourse.bass_utils import run_bass_kernel_spmd
from contextlib import ExitStack

f32 = mybir.dt.float32
fp16 = mybir.dt.float16

B, S, D = 2, 2048, 2048
HD = 128            # head dim
NHC = 4             # heads per core
HG = NHC * HD       # 512 weight cols per core
DT = D // 128       # 16 contraction tiles
SB = S // 128       # 16 seq blocks of 128
SC = S // 512       # 4 seq chunks of 512
SCALE = 1.0 / math.sqrt(HD)

# PT (transposed exp-scores) ragged layout: kblock kb covers q columns
# [kb*128, S), stored packed in one wide tile.
QA = [kb * 128 for kb in range(SB)]
WID = [S - QA[kb] for kb in range(SB)]
OFF = [0] * SB
for _kb in range(1, SB):
    OFF[_kb] = OFF[_kb - 1] + WID[_kb - 1]
PT_COLS = OFF[-1] + WID[-1]  # 20480

_CACHED = {}


def _build():
    nc = bacc.Bacc(trn_type="TRN2", target_bir_lowering=False, debug=False)
    xt_d = nc.dram_tensor("xt", [D, S], fp16, kind="ExternalInput").ap()
    wq_d = nc.dram_tensor("wq", [D, HG], fp16, kind="ExternalInput").ap()
    wk_d = nc.dram_tensor("wk", [D, HG], fp16, kind="ExternalInput").ap()
    wv_d = nc.dram_tensor("wv", [D, HG], fp16, kind="ExternalInput").ap()
    wo_d = nc.dram_tensor("wo", [HG, D], fp16, kind="ExternalInput").ap()
    mask_d = nc.dram_tensor("mask", [128, 512], fp16, kind="ExternalInput").ap()
    out_d = nc.dram_tensor("out", [S, D], f32, kind="ExternalOutput").ap()

    with tile.TileContext(nc) as tc, ExitStack() as ctx:
        # ---- long-lived pools
        v_p = ctx.enter_context(tc.tile_pool(name="v_p", bufs=SB))
        qt_p = ctx.enter_context(tc.tile_pool(name="qt_p", bufs=NHC))
        kt_p = ctx.enter_context(tc.tile_pool(name="kt_p", bufs=NHC))
        ctxt_p = ctx.enter_context(tc.tile_pool(name="ctxt_p", bufs=NHC))
        const_p = ctx.enter_context(tc.tile_pool(name="const_p", bufs=1))

        mask_sb = const_p.tile([128, 512], fp16, tag="mask_sb")
        nc.sync.dma_start(out=mask_sb[:], in_=mask_d[:])
        ident = const_p.tile([128, 128], fp16, tag="ident")
        masks.make_identity(nc, ident[:])

        pt0_p = ctx.enter_context(tc.tile_pool(name="pt0_p", bufs=1))
        wo_p = ctx.enter_context(tc.tile_pool(name="wo_p", bufs=NHC))

        # xT resident fp16 (projections only): rhs of Q/K, lhsT of V
        xt_scope = ExitStack()
        xt_p = xt_scope.enter_context(tc.tile_pool(name="xt_p", bufs=DT))
        xt = [None] * DT

        # score/psum pools shared by hoisted scores(0) and phase 3
        pss_p = ctx.enter_context(tc.tile_pool(name="pss_p", bufs=4, space="PSUM"))

        qt_tiles, kt_tiles = [], []
        ctxt_tiles = []

        def load_w(h, w_d, w_pool, wtag):
            wts = []
            for dt_ in range(DT):
                wt = w_pool.tile([128, 128], fp16, tag=wtag)
                nc.sync.dma_start(
                    out=wt[:],
                    in_=w_d[dt_ * 128:(dt_ + 1) * 128,
                            h * 128:(h + 1) * 128])
                wts.append(wt)
            return wts

        def qk_head(h, wq_p, wk_p, psqk_p, pre=None):
            for (w_d, w_pool, dst_pool, dst_list, wtag) in (
                    (wq_d, wq_p, qt_p, qt_tiles, "wqh"),
                    (wk_d, wk_p, kt_p, kt_tiles, "wkh")):
                dst = dst_pool.tile([128, S], fp16, tag=f"{wtag}dst")
                if pre is not None and wtag in pre:
                    wts = pre[wtag]
                else:
                    wts = load_w(h, w_d, w_pool, wtag)
                for sc_ in range(SC):
                    ps = psqk_p.tile([128, 512], f32, tag="psqk")
                    for dt_ in range(DT):
                        nc.tensor.matmul(
                            ps[:], wts[dt_][:],
                            xt[dt_][:, sc_ * 512:(sc_ + 1) * 512],
                            start=(dt_ == 0), stop=(dt_ == DT - 1))
                    nc.scalar.copy(dst[:, sc_ * 512:(sc_ + 1) * 512], ps[:])
                dst_list.append(dst)

        def scores_head(h, pool):
            qt_h, kt_h = qt_tiles[h], kt_tiles[h]
            pt = pool.tile([128, PT_COLS], fp16, tag="pt")
            for kb in range(SB):
                qa = QA[kb]
                qc = qa
                while qc < S:
                    w = min(512 - (qc % 512), S - qc)
                    pss = pss_p.tile([128, 512], f32, tag="pss")
                    nc.tensor.matmul(
                        pss[:, 0:w], kt_h[:, kb * 128:(kb + 1) * 128],
                        qt_h[:, qc:qc + w], start=True, stop=True)
                    dst = pt[:, OFF[kb] + (qc - qa):OFF[kb] + (qc - qa) + w]
                    nc.scalar.activation(
                        dst, pss[:, 0:w], mybir.ActivationFunctionType.Exp,
                        scale=SCALE)
                    if qc == qa:
                        wm = min(w, 128)
                        nc.vector.tensor_mul(
                            dst[:, 0:wm], dst[:, 0:wm], mask_sb[:, 0:wm])
                    qc += w
            return pt

        # ---- QK projections, with head-0 scores hoisted in between.
        # Head-0 weights DMA first so the first chain isn't gated on the
        # whole 8MB xT stream; xT tile DMAs interleave after.
        with tc.tile_pool(name="wq_p", bufs=18) as wq_p, \
             tc.tile_pool(name="wk_p", bufs=18) as wk_p, \
             tc.tile_pool(name="psqk_p", bufs=4, space="PSUM") as psqk_p:
            for i in range(DT):
                xti = xt_p.tile([128, S], fp16, tag="xt")
                nc.sync.dma_start(out=xti[:], in_=xt_d[i * 128:(i + 1) * 128, :])
                xt[i] = xti
            qk_head(0, wq_p, wk_p, psqk_p)
            pt0 = scores_head(0, pt0_p)
            for h in range(1, NHC):
                qk_head(h, wq_p, wk_p, psqk_p)

        # ---- V projection (natural layout [kpos, hd]), all 4 heads
        v_tiles = []
        with tc.tile_pool(name="wv_p", bufs=DT) as wv_p:
            wv = []
            for i in range(DT):
                t = wv_p.tile([128, HG], fp16, tag="wv")
                nc.sync.dma_start(out=t[:], in_=wv_d[i * 128:(i + 1) * 128, :])
                wv.append(t)
            with tc.tile_pool(name="psv_p", bufs=3, space="PSUM") as psv_p:
                for kb in range(SB):
                    psv = psv_p.tile([128, HG], f32, tag="psv")
                    for dt_ in range(DT):
                        nc.tensor.matmul(
                            psv[:], xt[dt_][:, kb * 128:(kb + 1) * 128], wv[dt_][:],
                            start=(dt_ == 0), stop=(dt_ == DT - 1))
                    vt = v_p.tile([128, NHC * 130], fp16, tag="vt")
                    vr = vt[:].rearrange("p (h c) -> p h c", c=130)
                    pr = psv[:].rearrange("p (h c) -> p h c", c=128)
                    nc.scalar.copy(vr[:, :, 0:128], pr[:, :, :])
                    nc.vector.memset(vr[:, :, 128:129], 1.0)
                    v_tiles.append(vt)

        xt_scope.close()

        # ---- phase 3: attention (scores h+1 pipelined ahead of PV h)
        with tc.tile_pool(name="psc_p", bufs=4, space="PSUM") as psc_p, \
             tc.tile_pool(name="pt_p", bufs=2) as pt_p, \
             tc.tile_pool(name="small_p", bufs=8) as small_p, \
             tc.tile_pool(name="stage_p", bufs=8) as stage_p:

            def pv_head(h, pt):
                ctxt_h = ctxt_p.tile([128, S], fp16, tag="ctxt")
                ctxt_tiles.append(ctxt_h)
                for qb in range(SB):
                    psc = psc_p.tile([128, 130], f32, tag="psc")
                    for kb in range(qb + 1):
                        lhs = pt[:, OFF[kb] + (qb - kb) * 128:
                                 OFF[kb] + (qb - kb) * 128 + 128]
                        nc.tensor.matmul(
                            psc[:, 0:129], lhs,
                            v_tiles[kb][:, h * 130:h * 130 + 129],
                            start=(kb == 0), stop=(kb == qb))
                    r = small_p.tile([128, 1], f32, tag="recip")
                    nc.vector.reciprocal(r[:], psc[:, 128:129])
                    cn = stage_p.tile([128, 128], fp16, tag="cn")
                    nc.vector.tensor_scalar_mul(cn[:], psc[:, 0:128], r[:])
                    nc.sync.dma_start(
                        out=ctxt_h[:, qb * 128:(qb + 1) * 128],
                        in_=cn[:], transpose=True)

            pts = {0: pt0}
            for h in range(NHC):
                if h + 1 < NHC:
                    pts[h + 1] = scores_head(h + 1, pt_p)
                pv_head(h, pts.pop(h))
                if h == 0:
                    # wo prefetch overlaps remaining attention work
                    wo = []
                    for hh in range(NHC):
                        wt_o = wo_p.tile([128, D], fp16, tag="wo")
                        nc.sync.dma_start(
                            out=wt_o[:], in_=wo_d[hh * 128:(hh + 1) * 128, :])
                        wo.append(wt_o)


        # ---- phase 4: out projection (wo prefetched above)
        with tc.tile_pool(name="pso_p", bufs=3, space="PSUM") as pso_p, \
             tc.tile_pool(name="ost_p", bufs=4) as ost_p:
            for st in range(SB):
                for dc in range(SC):
                    pso = pso_p.tile([128, 512], f32, tag="pso")
                    for hh in range(NHC):
                        nc.tensor.matmul(
                            pso[:], ctxt_tiles[hh][:, st * 128:(st + 1) * 128],
                            wo[hh][:, dc * 512:(dc + 1) * 512],
                            start=(hh == 0), stop=(hh == NHC - 1))
                    ob = ost_p.tile([128, 512], f32, tag="ob")
                    nc.scalar.copy(ob[:], pso[:])
                    nc.sync.dma_start(
                        out=out_d[st * 128:(st + 1) * 128, dc * 512:(dc + 1) * 512],
                        in_=ob[:])

    nc.compile()
    return nc


def _make_mask():
    kl = np.arange(128)[:, None]
    ql = np.arange(512)[None, :]
    return (ql >= kl).astype(np.float16)


def kernel(x, wq, wk, wv, wo, bo):
    if "nc" not in _CACHED:
        _CACHED["nc"] = _build()
    nc = _CACHED["nc"]
    mask = _make_mask()
    in_maps = []
    for c in range(8):
        b = c // 4
        hs = (c % 4) * HG
        xt = np.ascontiguousarray(np.asarray(x[b], dtype=np.float32).T).astype(
            np.float16)
        in_maps.append({
            "xt": xt,
            "wq": np.ascontiguousarray(np.asarray(wq[:, hs:hs + HG], np.float32)).astype(np.float16),
            "wk": np.ascontiguousarray(np.asarray(wk[:, hs:hs + HG], np.float32)).astype(np.float16),
            "wv": np.ascontiguousarray(np.asarray(wv[:, hs:hs + HG], np.float32)).astype(np.float16),
            "wo": np.ascontiguousarray(np.asarray(wo[hs:hs + HG, :], np.float32)).astype(np.float16),
            "mask": mask,
        })
    res = None
    for attempt in range(3):
        try:
            res = run_bass_kernel_spmd(nc, in_maps, core_ids=list(range(8)))
            break
        except Exception:
            if attempt == 2:
                raise
            import time as _time
            _time.sleep(5.0)
    out = np.zeros((B, S, D), dtype=np.float32)
    for b in range(B):
        acc = np.zeros((S, D), dtype=np.float32)
        for c in range(4 * b, 4 * b + 4):
            acc += res.results[c]["out"]
        out[b] = acc + np.asarray(bo, np.float32)[None, :]
    return out
